# revision 16
# baseline (speedup 1.0000x reference)
"""GQA causal attention (B=2, S=2048, HID=2048, H=32, HKV=8, D=128) on 8 TRN2
NeuronCores.

Sharding: tensor-parallel over heads for QKV+attention (core c owns kv head c
and q heads 4c..4c+3), then an AllToAll switches to sequence-parallel for
o_proj (core c computes the full hidden dim for global s-chunk c). The A2A
moves 8x less data than an AllGather and needs no per-core dynamic slicing.
It is split into two collectives (head pairs) so comm overlaps attention
compute of the remaining heads and the first half of o_proj.

Device pipeline (bf16 compute, fp32 PSUM accumulation):
  1. Feature-major projections: Q^T/K^T/V^T = W^T h^T, h^T streamed via two
     DMA queues (sync+scalar) in large pieces so the PE never starves.
  2. RoPE as  x*cos_dup + swap_halves(x)*sin_signed  - the rotate-half is a
     pure partition swap done by idle gpsimd SWDGE DMAs (the sign lives in the
     host-prepared sin table); cross-partition DVE ops are illegal.
  3. Transposed flash attention, both batches in lockstep per (head, qchunk):
     score matmuls for b=0/b=1 land in the two banks of one [128,2,512] PSUM
     pair, ONE exp ACTIVATE covers both (halves ScalarE instruction overhead,
     which is the attention-phase bottleneck), causal 0/1 mask multiplies only
     the 128 diagonal columns of both batches at once, denominators are
     accumulated on the DVE (og += pT) and reduced with a single ones-matmul
     per (b, qchunk) - 4 den matmuls per head-batch instead of 22.
  4. Two AllToAlls (heads 0-1, then 2-3) exchange attn-out^T blocks.
  5. o_proj: out^T[hid, my_s_chunk] accumulated over all 32 feature tiles
     (Wo host-permuted into A2A block order, streamed), fp32 out.
Host reassembles the 8 sequence chunks and transposes back.
"""

import os

import numpy as np
import ml_dtypes

from concourse import bacc, mybir
import concourse.tile as tile
from concourse.bass_utils import run_bass_kernel_spmd

N_CORES = 8
B, S, HID = 2, 2048, 2048
H, HKV, D = 32, 8, 128
QH = H // HKV          # q heads per core
SG = B * S             # 4096 global sequence
NSC = SG // 512        # 8 s-chunks of 512
NKT = HID // 128       # 16 hid k-tiles
NFT = (H * D) // 128   # 32 o_proj contraction tiles

BF = mybir.dt.bfloat16
F32 = mybir.dt.float32
AF = mybir.ActivationFunctionType

_CACHE = {}
LAST_EXEC_NS = None
LAST_TRACE = None


def _build():
    nc = bacc.Bacc("TRN2", num_devices=N_CORES)

    hT_e = nc.declare_dram_parameter("hT", [HID, SG], BF, isOutput=False)
    wq_e = nc.declare_dram_parameter("wq", [HID, QH * D], BF, isOutput=False)
    wk_e = nc.declare_dram_parameter("wk", [HID, D], BF, isOutput=False)
    wv_e = nc.declare_dram_parameter("wv", [HID, D], BF, isOutput=False)
    # Wo host-permuted into per-(half, hid_t) contiguous [128, 16*128]
    # blocks in the order o_proj consumes them (few-descriptor DMAs).
    wo_e = nc.declare_dram_parameter("wo", [2 * NKT * 128, 16 * 128], BF,
                                     isOutput=False)
    cd_e = nc.declare_dram_parameter("c_dup", [D, SG], BF, isOutput=False)
    sd_e = nc.declare_dram_parameter("s_dup", [D, SG], BF, isOutput=False)
    id_e = nc.declare_dram_parameter("ident", [D, D], BF, isOutput=False)
    # duplicated diagonal mask: masks[kk, b*128+c] = (kk <= c)
    mk_e = nc.declare_dram_parameter("masks", [128, 256], BF, isOutput=False)
    outT_e = nc.declare_dram_parameter("outT", [HID, 512], F32, isOutput=True)

    with tile.TileContext(nc) as tc:
        with (
            tc.tile_pool(name="cst", bufs=1) as cst,
            tc.tile_pool(name="sb", bufs=2) as sb,
            tc.tile_pool(name="agp", bufs=1) as agp,
            tc.tile_pool(name="dram", bufs=1, space="DRAM") as dram,
        ):
            tril2 = cst.tile([128, 2, 128], BF, tag="tril2")
            nc.sync.dma_start(tril2[:], mk_e[:].rearrange("p (b c) -> p b c", b=2))
            ones_mat = cst.tile([128, 128], BF, tag="ones_mat")
            nc.gpsimd.memset(ones_mat[:], 1.0)

            qr = cst.tile([128, QH * SG], BF, tag="qr")
            kr = cst.tile([128, SG], BF, tag="kr")
            v_seq = cst.tile([128, SG], BF, tag="v_seq")

            # A2A bounce buffers: shard j = rows [j*256, (j+1)*256) =
            # (2 heads x 128d, s-chunk j's 512 cols).
            a2a_in = [
                dram.tile([8 * 256, 512], BF, name=f"a2ain{i}", tag=f"a2ain{i}")
                for i in (0, 1)
            ]
            a2a_out = [
                dram.tile([8 * 256, 512], BF, name=f"a2aout{i}", tag=f"a2aout{i}")
                for i in (0, 1)
            ]

            # ---- phase 1: projections + rope + V transpose ----
            with tc.tile_pool(name="p1", bufs=1) as p1, \
                 tc.tile_pool(name="htp", bufs=3) as htp, \
                 tc.tile_pool(name="pmm", bufs=3, space="PSUM") as pmm, \
                 tc.tile_pool(name="ptp", bufs=2, space="PSUM") as ptp:
                # wq on the sync HWDGE queue, the first h^T chunk on the
                # scalar HWDGE queue - two independent FIFOs so the first
                # projection chain streams as pieces land on both.
                wq_sb = p1.tile([128, NKT, QH * D], BF, tag="wq_sb")
                ht0 = htp.tile([128, NKT, 512], BF, tag="ht")
                # kt-granular pieces for the first 8 kt (lets the first
                # chain start within ~2us), coarser 4-kt pieces after.
                pieces = [(k, 1) for k in range(8)] + [(8, 4), (12, 4)]
                for k0, nk in pieces:
                    nc.sync.dma_start(
                        wq_sb[:, k0 : k0 + nk, :],
                        wq_e[k0 * 128 : (k0 + nk) * 128, :].rearrange(
                            "(kt p) f -> p kt f", p=128
                        ),
                    )
                    nc.scalar.dma_start(
                        ht0[:, k0 : k0 + nk, :],
                        hT_e[k0 * 128 : (k0 + nk) * 128, 0:512].rearrange(
                            "(kt p) s -> p kt s", p=128
                        ),
                    )
                wk_sb = p1.tile([128, NKT, D], BF, tag="wk_sb")
                nc.sync.dma_start(
                    wk_sb[:], wk_e[:].rearrange("(kt p) f -> p kt f", p=128)
                )
                wv_sb = p1.tile([128, NKT, D], BF, tag="wv_sb")
                nc.sync.dma_start(
                    wv_sb[:], wv_e[:].rearrange("(kt p) f -> p kt f", p=128)
                )
                ident = p1.tile([D, D], BF, tag="ident")
                nc.sync.dma_start(ident[:], id_e[:])
                # rope tables via the gpsimd SWDGE queue (idle until rope)
                c_d = p1.tile([D, SG], BF, tag="c_d")
                nc.gpsimd.dma_start(c_d[:], cd_e[:])
                s_d = p1.tile([D, SG], BF, tag="s_d")
                nc.gpsimd.dma_start(s_d[:], sd_e[:])

                # rope/V-transpose for tile i are emitted AFTER projection
                # chain i+1 so their PE ops never wait on the ACT evacuation.
                def finish_tile(sc, ft, xb):
                    if ft < QH + 1:  # rope for q heads and k
                        # rotate-half = partition swap via idle gpsimd SWDGE
                        # (sin table sign-folded on host)
                        sh = sb.tile([128, 512], BF, tag="sh", bufs=3)
                        nc.gpsimd.dma_start(sh[0:64, :], xb[64:128, :])
                        nc.gpsimd.dma_start(sh[64:128, :], xb[0:64, :])
                        if ft < QH:
                            dest = qr[
                                :, ft * SG + sc * 512 : ft * SG + sc * 512 + 512
                            ]
                        else:
                            dest = kr[:, sc * 512 : sc * 512 + 512]
                        cs = c_d[:, sc * 512 : (sc + 1) * 512]
                        ss = s_d[:, sc * 512 : (sc + 1) * 512]
                        nc.vector.tensor_mul(dest, xb[:], cs)
                        rtmp = sb.tile([128, 512], BF, tag="rtmp")
                        nc.vector.tensor_mul(rtmp[:], sh[:], ss)
                        nc.vector.tensor_add(dest, dest, rtmp[:])
                    else:  # v: transpose to seq-major
                        for j in range(4):
                            tp = ptp.tile([128, 128], BF, tag="tp")
                            nc.tensor.transpose(
                                tp[:], xb[:, j * 128 : (j + 1) * 128], ident[:]
                            )
                            g = sc * 4 + j
                            nc.vector.tensor_copy(
                                v_seq[:, g * 128 : (g + 1) * 128], tp[:]
                            )

                with nc.named_scope("proj"):
                    pending = None
                    for sc in range(NSC):
                        if sc == 0:
                            ht = ht0
                        else:
                            ht = htp.tile([128, NKT, 512], BF, tag="ht")
                            src = hT_e[:, sc * 512 : (sc + 1) * 512].rearrange(
                                "(kt p) s -> p kt s", p=128
                            )
                            # alternate HWDGE queues to halve per-queue load
                            if sc % 2:
                                nc.scalar.dma_start(ht[:], src)
                            else:
                                nc.sync.dma_start(ht[:], src)
                        for ft in range(QH + 2):  # 0..3 q heads, 4 k, 5 v
                            acc = pmm.tile([128, 512], F32, tag="mm")
                            for kt in range(NKT):
                                if ft < QH:
                                    lhsT = wq_sb[:, kt, ft * D : (ft + 1) * D]
                                elif ft == QH:
                                    lhsT = wk_sb[:, kt, :]
                                else:
                                    lhsT = wv_sb[:, kt, :]
                                nc.tensor.matmul(
                                    acc[:], lhsT, ht[:, kt, :],
                                    start=(kt == 0), stop=(kt == NKT - 1),
                                )
                            xb = sb.tile([128, 512], BF, tag="xb", bufs=4)
                            nc.scalar.activation(xb[:], acc[:], AF.Copy)
                            if pending is not None:
                                finish_tile(*pending)
                            pending = (sc, ft, xb)
                    finish_tile(*pending)

            # ---- phase 2: attention, b=0/b=1 in lockstep per (h, qc),
            # flattened into one software pipeline: score matmuls always run
            # 2 kt-tiles ahead ACROSS qc/h boundaries so the per-qc epilogue
            # (den->recip->normalize) never bubbles the PE (bubbles also
            # HAM-rethrottle the PE clock, doubling the cost).
            agts = []
            with nc.named_scope("attn"), \
                 tc.tile_pool(name="spair", bufs=2, space="PSUM") as spair, \
                 tc.tile_pool(name="accp", bufs=2, space="PSUM") as accp, \
                 tc.tile_pool(name="sba", bufs=1) as sba:

                stream = []
                for half in range(2):
                    for h in (2 * half, 2 * half + 1):
                        for qc in range(4):
                            nkt = 4 * qc + 4
                            for kt in range(nkt):
                                stream.append((half, h, qc, kt, nkt))

                def qoff(qc, kt):
                    j = kt - 4 * qc
                    return j * 128 if j > 0 else 0

                sptiles = {}

                def emit_score(j):
                    _, h, qc, kt, _ = stream[j]
                    o = qoff(qc, kt)
                    sp = spair.tile(
                        [128, 2, 512], F32, tag="sp", name=f"s_{h}_{qc}_{kt}"
                    )
                    for b in range(2):
                        qs = h * SG + b * S + qc * 512
                        nc.tensor.matmul(
                            sp[:, b, : 512 - o],
                            kr[:, b * S + kt * 128 : b * S + (kt + 1) * 128],
                            qr[:, qs + o : qs + 512],
                        )
                    sptiles[j] = sp

                state = {}
                emitted = 0
                for i, (half, h, qc, kt, nkt) in enumerate(stream):
                    while emitted < min(i + 2, len(stream)):
                        emit_score(emitted)
                        emitted += 1
                    sp = sptiles.pop(i)
                    o = qoff(qc, kt)
                    w = 512 - o
                    pT = sba.tile([128, 2, 512], BF, tag="pT", bufs=4)
                    # one exp covers both batches (2D free AP, 2 banks)
                    nc.scalar.activation(pT[:, :, :w], sp[:, :, :w], AF.Exp)
                    if kt - 4 * qc >= 0:
                        # only the 128 diagonal cols need the mask
                        nc.vector.tensor_mul(
                            pT[:, :, :128], pT[:, :, :128], tril2[:]
                        )
                    if kt == 0:
                        acc = accp.tile([128, 2, 512], F32, tag="accp")
                        og = sba.tile([128, 2, 512], BF, tag="og", bufs=2)
                        state[(h, qc)] = (acc, og)
                    acc, og = state[(h, qc)]
                    for b in range(2):
                        g = b * 16 + kt
                        nc.tensor.matmul(
                            acc[:, b, o:512],
                            v_seq[:, g * 128 : (g + 1) * 128],
                            pT[:, b, :w],
                            start=(kt == 0), stop=(kt == nkt - 1),
                        )
                    # denominator accumulation on the DVE (both b at once)
                    if kt == 0:
                        nc.vector.tensor_copy(og[:], pT[:])
                    else:
                        nc.vector.tensor_add(
                            og[:, :, o:512], og[:, :, o:512], pT[:, :, :w]
                        )
                    if kt != nkt - 1:
                        continue
                    # ---- (h, qc) epilogue ----
                    # den shares the spair ring (single-shot matmuls; the
                    # bank is reused by later scores after recip reads it)
                    den = spair.tile(
                        [128, 2, 512], F32, tag="sp", name=f"den_{h}_{qc}"
                    )
                    nc.tensor.matmul(den[:, 0, :], ones_mat[:], og[:, 0, :])
                    nc.tensor.matmul(den[:, 1, :], ones_mat[:], og[:, 1, :])
                    # den rows are identical (all-ones stationary) == already
                    # broadcast across partitions.
                    rb = sba.tile([128, 2, 512], F32, tag="rb", bufs=2)
                    nc.vector.reciprocal_approx_fast(rb[:], den[:])
                    ao = sba.tile([128, 2, 512], BF, tag="ao", bufs=3)
                    nc.vector.tensor_mul(ao[:], acc[:], rb[:])
                    hh = h % 2
                    for b in range(2):
                        sc = b * 4 + qc
                        nc.sync.dma_start(
                            a2a_in[half][
                                sc * 256 + hh * 128 : sc * 256 + (hh + 1) * 128, :
                            ],
                            ao[:, b, :],
                        )
                    if hh == 1 and qc == 3:  # last (h, qc) of this half
                        nc.gpsimd.collective_compute(
                            "AllToAll",
                            mybir.AluOpType.bypass,
                            replica_groups=[list(range(N_CORES))],
                            ins=[a2a_in[half].opt()],
                            outs=[a2a_out[half].opt()],
                        )
                        # a2a_out loads emitted HERE so they sit in the
                        # gpsimd FIFO before the next half's collective
                        # trigger (which blocks the queue while waiting on
                        # its own input writes).
                        agt = agp.tile([128, 16, 512], BF, tag=f"ag{half}")
                        for fq in range(4):
                            nc.gpsimd.dma_start(
                                agt[:, 4 * fq : 4 * fq + 4, :],
                                a2a_out[half][
                                    4 * fq * 128 : (4 * fq + 4) * 128, :
                                ].rearrange("(ft p) s -> p ft s", p=128),
                            )
                        agts.append(agt)

            # ---- phase 3: o_proj for my s-chunk, all hidden columns.
            # Two passes: pass 0 (features from A2A1) accumulates to SBUF
            # partials while A2A2 is still in flight; pass 1 adds the rest.
            with nc.named_scope("oproj"), \
                 tc.tile_pool(name="po", bufs=4, space="PSUM") as po, \
                 tc.tile_pool(name="wop", bufs=8) as wop, \
                 tc.tile_pool(name="prt", bufs=1) as prt:
                parts = []
                for half in range(2):
                    agt = agts[half]
                    for hid_t in range(NKT):  # 16 tiles of 128 hidden cols
                        wo_t = wop.tile([128, 16, 128], BF, tag="wo_t")
                        r0 = (half * NKT + hid_t) * 128
                        nc.sync.dma_start(
                            wo_t[:],
                            wo_e[r0 : r0 + 128, :].rearrange(
                                "p (ft c) -> p ft c", ft=16
                            ),
                        )
                        o_ps = po.tile([128, 512], F32, tag="mm")
                        for ft in range(16):
                            nc.tensor.matmul(
                                o_ps[:],
                                wo_t[:, ft, :],
                                agt[:, ft, :],
                                start=(ft == 0),
                                stop=(ft == 15),
                            )
                        if half == 0:
                            part = prt.tile(
                                [128, 512], F32, tag=f"part{hid_t}"
                            )
                            nc.scalar.activation(part[:], o_ps[:], AF.Copy)
                            parts.append(part)
                        else:
                            ob = sb.tile([128, 512], F32, tag="ob", bufs=3)
                            nc.vector.tensor_add(ob[:], o_ps[:], parts[hid_t][:])
                            nc.sync.dma_start(
                                outT_e[hid_t * 128 : (hid_t + 1) * 128, :], ob[:]
                            )

    nc.compile()
    return nc


def _prep(hidden_states, sin_table, cos_table, Wq, Wk, Wv, Wo):
    bf = ml_dtypes.bfloat16
    flat = np.asarray(hidden_states, np.float32).reshape(SG, HID)
    hT = np.ascontiguousarray(flat.T).astype(bf)

    cosT = np.asarray(cos_table, np.float32)[:, :64].T  # [64, S]
    sinT = np.asarray(sin_table, np.float32)[:, :64].T
    c_dup = np.tile(np.concatenate([cosT, cosT], 0), (1, B)).astype(bf)
    # sign-folded: rotate-half becomes a plain partition swap
    s_dup = np.tile(np.concatenate([-sinT, sinT], 0), (1, B)).astype(bf)

    ident = np.eye(D, dtype=np.float32).astype(bf)

    kk = np.arange(128)[:, None]
    cc = np.arange(128)[None, :]
    tri = (kk <= cc).astype(np.float32)
    masks = np.concatenate([tri, tri], axis=1).astype(bf)  # [128, 256]

    scale = np.float32(1.0 / np.sqrt(D))
    Wq = np.asarray(Wq, np.float32) * scale
    Wk = np.asarray(Wk, np.float32)
    Wv = np.asarray(Wv, np.float32)
    Wo = np.asarray(Wo, np.float32)

    # Permute Wo rows into the order o_proj consumes the A2A output blocks
    # (a2a1 blocks: (r, h in {0,1}); a2a2 blocks: (r, h in {2,3})), then
    # re-block so each (half, hid_t) wo_t tile is one contiguous
    # [128, 16*128] DRAM block: wo_r[half, hid_t, p, ft, c].
    Wo_b = Wo.reshape(H, D, HID)
    order = [4 * r + h for r in range(8) for h in (0, 1)] + [
        4 * r + h for r in range(8) for h in (2, 3)
    ]
    Wo_perm = Wo_b[order].reshape(H * D, HID)
    Wp = Wo_perm.reshape(2, 16, 128, NKT, 128)  # [half, ft, p, hid_t, c]
    Wo_r = np.ascontiguousarray(
        Wp.transpose(0, 3, 2, 1, 4).reshape(2 * NKT * 128, 16 * 128)
    ).astype(bf)

    in_maps = []
    for c in range(N_CORES):
        in_maps.append(
            {
                "hT": hT,
                "wq": np.ascontiguousarray(Wq[:, c * 512 : (c + 1) * 512]).astype(bf),
                "wk": np.ascontiguousarray(Wk[:, c * D : (c + 1) * D]).astype(bf),
                "wv": np.ascontiguousarray(Wv[:, c * D : (c + 1) * D]).astype(bf),
                "wo": Wo_r,
                "c_dup": c_dup,
                "s_dup": s_dup,
                "ident": ident,
                "masks": masks,
            }
        )
    return in_maps


def kernel(**inputs) -> np.ndarray:
    global LAST_EXEC_NS, LAST_TRACE
    if "nc" not in _CACHE:
        _CACHE["nc"] = _build()
    nc = _CACHE["nc"]

    in_maps = _prep(**inputs)
    res = run_bass_kernel_spmd(
        nc,
        in_maps,
        core_ids=list(range(N_CORES)),
        trace=bool(os.environ.get("BASS_TRACE")),
    )
    LAST_EXEC_NS = res.exec_time_ns
    LAST_TRACE = res.instructions_and_trace
    globals()["LAST_SCOPES"] = res.per_core_scope_times

    outT = np.concatenate(
        [np.asarray(res.results[c]["outT"], np.float32) for c in range(N_CORES)],
        axis=1,
    )  # [HID, SG]
    return np.ascontiguousarray(outT.T).reshape(B, S, HID)


# revision 21
# speedup vs baseline: 1.0060x; 1.0060x over previous
"""GQA causal attention (B=2, S=2048, HID=2048, H=32, HKV=8, D=128) on 8 TRN2
NeuronCores.

Sharding: tensor-parallel over heads for QKV+attention (core c owns kv head c
and q heads 4c..4c+3), then an AllToAll switches to sequence-parallel for
o_proj (core c computes the full hidden dim for global s-chunk c). The A2A
moves 8x less data than an AllGather and needs no per-core dynamic slicing.
It is split into two collectives (head pairs) so comm overlaps attention
compute of the remaining heads and the first half of o_proj.

Device pipeline (bf16 compute, fp32 PSUM accumulation):
  1. Feature-major projections: Q^T/K^T/V^T = W^T h^T, h^T streamed via two
     DMA queues (sync+scalar) in large pieces so the PE never starves.
  2. RoPE as  x*cos_dup + swap_halves(x)*sin_signed  - the rotate-half is a
     pure partition swap done by idle gpsimd SWDGE DMAs (the sign lives in the
     host-prepared sin table); cross-partition DVE ops are illegal.
  3. Transposed flash attention, both batches in lockstep per (head, qchunk):
     score matmuls for b=0/b=1 land in the two banks of one [128,2,512] PSUM
     pair, ONE exp ACTIVATE covers both (halves ScalarE instruction overhead,
     which is the attention-phase bottleneck), causal 0/1 mask multiplies only
     the 128 diagonal columns of both batches at once, denominators are
     accumulated on the DVE (og += pT) and reduced with a single ones-matmul
     per (b, qchunk) - 4 den matmuls per head-batch instead of 22.
  4. Two AllToAlls (heads 0-1, then 2-3) exchange attn-out^T blocks.
  5. o_proj: out^T[hid, my_s_chunk] accumulated over all 32 feature tiles
     (Wo host-permuted into A2A block order, streamed), fp32 out.
Host reassembles the 8 sequence chunks and transposes back.
"""

import os

import numpy as np
import ml_dtypes

from concourse import bacc, mybir
import concourse.tile as tile
from concourse.bass_utils import run_bass_kernel_spmd

N_CORES = 8
B, S, HID = 2, 2048, 2048
H, HKV, D = 32, 8, 128
QH = H // HKV          # q heads per core
SG = B * S             # 4096 global sequence
NSC = SG // 512        # 8 s-chunks of 512
NKT = HID // 128       # 16 hid k-tiles
NFT = (H * D) // 128   # 32 o_proj contraction tiles

BF = mybir.dt.bfloat16
F32 = mybir.dt.float32
AF = mybir.ActivationFunctionType

_CACHE = {}
LAST_EXEC_NS = None
LAST_TRACE = None


def _build():
    nc = bacc.Bacc("TRN2", num_devices=N_CORES)

    hT_e = nc.declare_dram_parameter("hT", [HID, SG], BF, isOutput=False)
    wq_e = nc.declare_dram_parameter("wq", [HID, QH * D], BF, isOutput=False)
    wk_e = nc.declare_dram_parameter("wk", [HID, D], BF, isOutput=False)
    wv_e = nc.declare_dram_parameter("wv", [HID, D], BF, isOutput=False)
    # Wo host-permuted into per-(half, hid_t) contiguous [128, 16*128]
    # blocks in the order o_proj consumes them (few-descriptor DMAs).
    wo_e = nc.declare_dram_parameter("wo", [2 * NKT * 128, 16 * 128], BF,
                                     isOutput=False)
    cd_e = nc.declare_dram_parameter("c_dup", [D, SG], BF, isOutput=False)
    sd_e = nc.declare_dram_parameter("s_dup", [D, SG], BF, isOutput=False)
    id_e = nc.declare_dram_parameter("ident", [D, D], BF, isOutput=False)
    # duplicated diagonal mask: masks[kk, b*128+c] = (kk <= c)
    mk_e = nc.declare_dram_parameter("masks", [128, 256], BF, isOutput=False)
    outT_e = nc.declare_dram_parameter("outT", [HID, 512], F32, isOutput=True)

    with tile.TileContext(nc) as tc:
        with (
            tc.tile_pool(name="cst", bufs=1) as cst,
            tc.tile_pool(name="sb", bufs=2) as sb,
            tc.tile_pool(name="agp", bufs=1) as agp,
            tc.tile_pool(name="wop", bufs=8) as wop,
            tc.tile_pool(name="dram", bufs=1, space="DRAM") as dram,
        ):
            tril2 = cst.tile([128, 2, 128], BF, tag="tril2")
            nc.sync.dma_start(tril2[:], mk_e[:].rearrange("p (b c) -> p b c", b=2))
            ones_mat = cst.tile([128, 128], BF, tag="ones_mat")
            nc.gpsimd.memset(ones_mat[:], 1.0)

            qr = cst.tile([128, QH * SG], BF, tag="qr")
            kr = cst.tile([128, SG], BF, tag="kr")
            v_seq = cst.tile([128, SG], BF, tag="v_seq")

            # A2A bounce buffers: shard j = rows [j*256, (j+1)*256) =
            # (2 heads x 128d, s-chunk j's 512 cols).
            a2a_in = [
                dram.tile([8 * 256, 512], BF, name=f"a2ain{i}", tag=f"a2ain{i}")
                for i in (0, 1)
            ]
            a2a_out = [
                dram.tile([8 * 256, 512], BF, name=f"a2aout{i}", tag=f"a2aout{i}")
                for i in (0, 1)
            ]

            # ---- phase 1: projections + rope + V transpose ----
            with tc.tile_pool(name="p1", bufs=1) as p1, \
                 tc.tile_pool(name="htp", bufs=2) as htp, \
                 tc.tile_pool(name="pmm", bufs=3, space="PSUM") as pmm, \
                 tc.tile_pool(name="ptp", bufs=2, space="PSUM") as ptp:
                # wq on the sync HWDGE queue, the first h^T chunk on the
                # scalar HWDGE queue - two independent FIFOs so the first
                # projection chain streams as pieces land on both.
                wq_sb = p1.tile([128, NKT, QH * D], BF, tag="wq_sb")
                ht0 = htp.tile([128, NKT, 512], BF, tag="ht")
                # kt-granular pieces for the first 8 kt (lets the first
                # chain start within ~2us), coarser 4-kt pieces after.
                # Pieces alternate between the two HWDGE queues so the
                # first chain streams at 2x the single-queue issue rate.
                pieces = [(k, 1) for k in range(8)] + [(8, 4), (12, 4)]
                for pi, (k0, nk) in enumerate(pieces):
                    qa = nc.sync if pi % 2 == 0 else nc.scalar
                    qb = nc.scalar if pi % 2 == 0 else nc.sync
                    qa.dma_start(
                        wq_sb[:, k0 : k0 + nk, :],
                        wq_e[k0 * 128 : (k0 + nk) * 128, :].rearrange(
                            "(kt p) f -> p kt f", p=128
                        ),
                    )
                    qb.dma_start(
                        ht0[:, k0 : k0 + nk, :],
                        hT_e[k0 * 128 : (k0 + nk) * 128, 0:512].rearrange(
                            "(kt p) s -> p kt s", p=128
                        ),
                    )
                wk_sb = p1.tile([128, NKT, D], BF, tag="wk_sb")
                nc.sync.dma_start(
                    wk_sb[:], wk_e[:].rearrange("(kt p) f -> p kt f", p=128)
                )
                wv_sb = p1.tile([128, NKT, D], BF, tag="wv_sb")
                nc.sync.dma_start(
                    wv_sb[:], wv_e[:].rearrange("(kt p) f -> p kt f", p=128)
                )
                ident = p1.tile([D, D], BF, tag="ident")
                nc.sync.dma_start(ident[:], id_e[:])
                # rope tables via the gpsimd SWDGE queue (idle until rope)
                c_d = p1.tile([D, SG], BF, tag="c_d")
                nc.gpsimd.dma_start(c_d[:], cd_e[:])
                s_d = p1.tile([D, SG], BF, tag="s_d")
                nc.gpsimd.dma_start(s_d[:], sd_e[:])

                # rope/V-transpose for tile i are emitted AFTER projection
                # chain i+1 so their PE ops never wait on the ACT evacuation.
                def finish_tile(sc, ft, xb):
                    if ft < QH + 1:  # rope for q heads and k
                        # rotate-half = partition swap via idle gpsimd SWDGE
                        # (sin table sign-folded on host)
                        sh = sb.tile([128, 512], BF, tag="sh", bufs=3)
                        nc.gpsimd.dma_start(sh[0:64, :], xb[64:128, :])
                        nc.gpsimd.dma_start(sh[64:128, :], xb[0:64, :])
                        if ft < QH:
                            dest = qr[
                                :, ft * SG + sc * 512 : ft * SG + sc * 512 + 512
                            ]
                        else:
                            dest = kr[:, sc * 512 : sc * 512 + 512]
                        cs = c_d[:, sc * 512 : (sc + 1) * 512]
                        ss = s_d[:, sc * 512 : (sc + 1) * 512]
                        nc.vector.tensor_mul(dest, xb[:], cs)
                        rtmp = sb.tile([128, 512], BF, tag="rtmp")
                        nc.vector.tensor_mul(rtmp[:], sh[:], ss)
                        nc.vector.tensor_add(dest, dest, rtmp[:])
                    else:  # v: transpose to seq-major
                        for j in range(4):
                            tp = ptp.tile([128, 128], BF, tag="tp")
                            nc.tensor.transpose(
                                tp[:], xb[:, j * 128 : (j + 1) * 128], ident[:]
                            )
                            g = sc * 4 + j
                            nc.vector.tensor_copy(
                                v_seq[:, g * 128 : (g + 1) * 128], tp[:]
                            )

                with nc.named_scope("proj"):
                    pending = None
                    for sc in range(NSC):
                        if sc == 0:
                            ht = ht0
                        else:
                            ht = htp.tile([128, NKT, 512], BF, tag="ht")
                            src = hT_e[:, sc * 512 : (sc + 1) * 512].rearrange(
                                "(kt p) s -> p kt s", p=128
                            )
                            # alternate HWDGE queues to halve per-queue load
                            if sc % 2:
                                nc.scalar.dma_start(ht[:], src)
                            else:
                                nc.sync.dma_start(ht[:], src)
                        for ft in range(QH + 2):  # 0..3 q heads, 4 k, 5 v
                            acc = pmm.tile([128, 512], F32, tag="mm")
                            for kt in range(NKT):
                                if ft < QH:
                                    lhsT = wq_sb[:, kt, ft * D : (ft + 1) * D]
                                elif ft == QH:
                                    lhsT = wk_sb[:, kt, :]
                                else:
                                    lhsT = wv_sb[:, kt, :]
                                nc.tensor.matmul(
                                    acc[:], lhsT, ht[:, kt, :],
                                    start=(kt == 0), stop=(kt == NKT - 1),
                                )
                            xb = sb.tile([128, 512], BF, tag="xb", bufs=4)
                            nc.scalar.activation(xb[:], acc[:], AF.Copy)
                            if pending is not None:
                                finish_tile(*pending)
                            pending = (sc, ft, xb)
                    finish_tile(*pending)

            # Prefetch the first 8 o_proj weight tiles NOW: these sync-queue
            # loads sit AHEAD of the attention a2a_in writes in the FIFO, so
            # they land during attention instead of head-of-line blocking
            # behind the last epilogue (only 8 = ring capacity; a 9th would
            # deadlock the queue against oproj progress).
            def load_wo(half, hid_t):
                wo_t = wop.tile([128, 16, 128], BF, tag="wo_t")
                r0 = (half * NKT + hid_t) * 128
                nc.sync.dma_start(
                    wo_t[:],
                    wo_e[r0 : r0 + 128, :].rearrange("p (ft c) -> p ft c", ft=16),
                )
                return wo_t

            wo_pre = [load_wo(0, hid_t) for hid_t in range(8)]

            # ---- phase 2: attention, b=0/b=1 in lockstep per (h, qc),
            # flattened into one software pipeline: score matmuls always run
            # 2 kt-tiles ahead ACROSS qc/h boundaries so the per-qc epilogue
            # (den->recip->normalize) never bubbles the PE (bubbles also
            # HAM-rethrottle the PE clock, doubling the cost).
            agts = []
            with nc.named_scope("attn"), \
                 tc.tile_pool(name="spair", bufs=2, space="PSUM") as spair, \
                 tc.tile_pool(name="accp", bufs=2, space="PSUM") as accp, \
                 tc.tile_pool(name="sba", bufs=1) as sba:

                stream = []
                for half in range(2):
                    for h in (2 * half, 2 * half + 1):
                        for qc in range(4):
                            nkt = 4 * qc + 4
                            for kt in range(nkt):
                                stream.append((half, h, qc, kt, nkt))

                def qoff(qc, kt):
                    j = kt - 4 * qc
                    return j * 128 if j > 0 else 0

                sptiles = {}

                def emit_score(j):
                    _, h, qc, kt, _ = stream[j]
                    o = qoff(qc, kt)
                    sp = spair.tile(
                        [128, 2, 512], F32, tag="sp", name=f"s_{h}_{qc}_{kt}"
                    )
                    for b in range(2):
                        qs = h * SG + b * S + qc * 512
                        nc.tensor.matmul(
                            sp[:, b, : 512 - o],
                            kr[:, b * S + kt * 128 : b * S + (kt + 1) * 128],
                            qr[:, qs + o : qs + 512],
                        )
                    sptiles[j] = sp

                state = {}
                emitted = 0
                for i, (half, h, qc, kt, nkt) in enumerate(stream):
                    while emitted < min(i + 2, len(stream)):
                        emit_score(emitted)
                        emitted += 1
                    sp = sptiles.pop(i)
                    o = qoff(qc, kt)
                    w = 512 - o
                    pT = sba.tile([128, 2, 512], BF, tag="pT", bufs=4)
                    # one exp covers both batches (2D free AP, 2 banks)
                    nc.scalar.activation(pT[:, :, :w], sp[:, :, :w], AF.Exp)
                    if kt - 4 * qc >= 0:
                        # only the 128 diagonal cols need the mask
                        nc.vector.tensor_mul(
                            pT[:, :, :128], pT[:, :, :128], tril2[:]
                        )
                    if kt == 0:
                        acc = accp.tile([128, 2, 512], F32, tag="accp")
                        og = sba.tile([128, 2, 512], BF, tag="og", bufs=2)
                        state[(h, qc)] = (acc, og)
                    acc, og = state[(h, qc)]
                    for b in range(2):
                        g = b * 16 + kt
                        nc.tensor.matmul(
                            acc[:, b, o:512],
                            v_seq[:, g * 128 : (g + 1) * 128],
                            pT[:, b, :w],
                            start=(kt == 0), stop=(kt == nkt - 1),
                        )
                    # denominator accumulation on the DVE (both b at once)
                    if kt == 0:
                        nc.vector.tensor_copy(og[:], pT[:])
                    else:
                        nc.vector.tensor_add(
                            og[:, :, o:512], og[:, :, o:512], pT[:, :, :w]
                        )
                    if kt != nkt - 1:
                        continue
                    # ---- (h, qc) epilogue ----
                    # den shares the spair ring (single-shot matmuls; the
                    # bank is reused by later scores after recip reads it)
                    den = spair.tile(
                        [128, 2, 512], F32, tag="sp", name=f"den_{h}_{qc}"
                    )
                    nc.tensor.matmul(den[:, 0, :], ones_mat[:], og[:, 0, :])
                    nc.tensor.matmul(den[:, 1, :], ones_mat[:], og[:, 1, :])
                    # den rows are identical (all-ones stationary) == already
                    # broadcast across partitions.
                    rb = sba.tile([128, 2, 512], F32, tag="rb", bufs=2)
                    nc.vector.reciprocal_approx_fast(rb[:], den[:])
                    ao = sba.tile([128, 2, 512], BF, tag="ao", bufs=3)
                    nc.vector.tensor_mul(ao[:], acc[:], rb[:])
                    hh = h % 2
                    for b in range(2):
                        sc = b * 4 + qc
                        nc.sync.dma_start(
                            a2a_in[half][
                                sc * 256 + hh * 128 : sc * 256 + (hh + 1) * 128, :
                            ],
                            ao[:, b, :],
                        )
                    if hh == 1 and qc == 3:  # last (h, qc) of this half
                        nc.gpsimd.collective_compute(
                            "AllToAll",
                            mybir.AluOpType.bypass,
                            replica_groups=[list(range(N_CORES))],
                            ins=[a2a_in[half].opt()],
                            outs=[a2a_out[half].opt()],
                        )
                        # a2a_out loads emitted HERE so they sit in the
                        # gpsimd FIFO before the next half's collective
                        # trigger (which blocks the queue while waiting on
                        # its own input writes).
                        agt = agp.tile([128, 16, 512], BF, tag=f"ag{half}")
                        for fq in range(4):
                            nc.gpsimd.dma_start(
                                agt[:, 4 * fq : 4 * fq + 4, :],
                                a2a_out[half][
                                    4 * fq * 128 : (4 * fq + 4) * 128, :
                                ].rearrange("(ft p) s -> p ft s", p=128),
                            )
                        agts.append(agt)

            # ---- phase 3: o_proj for my s-chunk, all hidden columns.
            # Two passes: pass 0 (features from A2A1) accumulates to SBUF
            # partials while A2A2 is still in flight; pass 1 adds the rest.
            with nc.named_scope("oproj"), \
                 tc.tile_pool(name="po", bufs=4, space="PSUM") as po, \
                 tc.tile_pool(name="prt", bufs=1) as prt:
                parts = []
                for half in range(2):
                    agt = agts[half]
                    for hid_t in range(NKT):  # 16 tiles of 128 hidden cols
                        if half == 0 and hid_t < 8:
                            wo_t = wo_pre[hid_t]
                        else:
                            wo_t = load_wo(half, hid_t)
                        o_ps = po.tile([128, 512], F32, tag="mm")
                        for ft in range(16):
                            nc.tensor.matmul(
                                o_ps[:],
                                wo_t[:, ft, :],
                                agt[:, ft, :],
                                start=(ft == 0),
                                stop=(ft == 15),
                            )
                        if half == 0:
                            part = prt.tile(
                                [128, 512], F32, tag=f"part{hid_t}"
                            )
                            nc.scalar.activation(part[:], o_ps[:], AF.Copy)
                            parts.append(part)
                        else:
                            ob = sb.tile([128, 512], F32, tag="ob", bufs=3)
                            nc.vector.tensor_add(ob[:], o_ps[:], parts[hid_t][:])
                            nc.sync.dma_start(
                                outT_e[hid_t * 128 : (hid_t + 1) * 128, :], ob[:]
                            )

    nc.compile()
    return nc


def _prep(hidden_states, sin_table, cos_table, Wq, Wk, Wv, Wo):
    bf = ml_dtypes.bfloat16
    flat = np.asarray(hidden_states, np.float32).reshape(SG, HID)
    hT = np.ascontiguousarray(flat.T).astype(bf)

    cosT = np.asarray(cos_table, np.float32)[:, :64].T  # [64, S]
    sinT = np.asarray(sin_table, np.float32)[:, :64].T
    c_dup = np.tile(np.concatenate([cosT, cosT], 0), (1, B)).astype(bf)
    # sign-folded: rotate-half becomes a plain partition swap
    s_dup = np.tile(np.concatenate([-sinT, sinT], 0), (1, B)).astype(bf)

    ident = np.eye(D, dtype=np.float32).astype(bf)

    kk = np.arange(128)[:, None]
    cc = np.arange(128)[None, :]
    tri = (kk <= cc).astype(np.float32)
    masks = np.concatenate([tri, tri], axis=1).astype(bf)  # [128, 256]

    scale = np.float32(1.0 / np.sqrt(D))
    Wq = np.asarray(Wq, np.float32) * scale
    Wk = np.asarray(Wk, np.float32)
    Wv = np.asarray(Wv, np.float32)
    Wo = np.asarray(Wo, np.float32)

    # Permute Wo rows into the order o_proj consumes the A2A output blocks
    # (a2a1 blocks: (r, h in {0,1}); a2a2 blocks: (r, h in {2,3})), then
    # re-block so each (half, hid_t) wo_t tile is one contiguous
    # [128, 16*128] DRAM block: wo_r[half, hid_t, p, ft, c].
    Wo_b = Wo.reshape(H, D, HID)
    order = [4 * r + h for r in range(8) for h in (0, 1)] + [
        4 * r + h for r in range(8) for h in (2, 3)
    ]
    Wo_perm = Wo_b[order].reshape(H * D, HID)
    Wp = Wo_perm.reshape(2, 16, 128, NKT, 128)  # [half, ft, p, hid_t, c]
    Wo_r = np.ascontiguousarray(
        Wp.transpose(0, 3, 2, 1, 4).reshape(2 * NKT * 128, 16 * 128)
    ).astype(bf)

    in_maps = []
    for c in range(N_CORES):
        in_maps.append(
            {
                "hT": hT,
                "wq": np.ascontiguousarray(Wq[:, c * 512 : (c + 1) * 512]).astype(bf),
                "wk": np.ascontiguousarray(Wk[:, c * D : (c + 1) * D]).astype(bf),
                "wv": np.ascontiguousarray(Wv[:, c * D : (c + 1) * D]).astype(bf),
                "wo": Wo_r,
                "c_dup": c_dup,
                "s_dup": s_dup,
                "ident": ident,
                "masks": masks,
            }
        )
    return in_maps


def kernel(**inputs) -> np.ndarray:
    global LAST_EXEC_NS, LAST_TRACE
    if "nc" not in _CACHE:
        _CACHE["nc"] = _build()
    nc = _CACHE["nc"]

    in_maps = _prep(**inputs)
    res = run_bass_kernel_spmd(
        nc,
        in_maps,
        core_ids=list(range(N_CORES)),
        trace=bool(os.environ.get("BASS_TRACE")),
    )
    LAST_EXEC_NS = res.exec_time_ns
    LAST_TRACE = res.instructions_and_trace
    globals()["LAST_SCOPES"] = res.per_core_scope_times

    outT = np.concatenate(
        [np.asarray(res.results[c]["outT"], np.float32) for c in range(N_CORES)],
        axis=1,
    )  # [HID, SG]
    return np.ascontiguousarray(outT.T).reshape(B, S, HID)


# revision 25
# speedup vs baseline: 1.0376x; 1.0314x over previous
"""GQA causal attention (B=2, S=2048, HID=2048, H=32, HKV=8, D=128) on 8 TRN2
NeuronCores.

Sharding: tensor-parallel over heads for QKV+attention (core c owns kv head c
and q heads 4c..4c+3), then an AllToAll switches to sequence-parallel for
o_proj (core c computes the full hidden dim for global s-chunk c). The A2A
moves 8x less data than an AllGather and needs no per-core dynamic slicing.
It is split into two collectives (head pairs) so comm overlaps attention
compute of the remaining heads and the first half of o_proj.

Device pipeline (bf16 compute, fp32 PSUM accumulation):
  1. Feature-major projections: Q^T/K^T/V^T = W^T h^T, h^T streamed via two
     DMA queues (sync+scalar) in large pieces so the PE never starves.
  2. RoPE as  x*cos_dup + swap_halves(x)*sin_signed  - the rotate-half is a
     pure partition swap done by idle gpsimd SWDGE DMAs (the sign lives in the
     host-prepared sin table); cross-partition DVE ops are illegal.
  3. Transposed flash attention, both batches in lockstep per (head, qchunk):
     score matmuls for b=0/b=1 land in the two banks of one [128,2,512] PSUM
     pair, ONE exp ACTIVATE covers both (halves ScalarE instruction overhead,
     which is the attention-phase bottleneck), causal 0/1 mask multiplies only
     the 128 diagonal columns of both batches at once, denominators are
     accumulated on the DVE (og += pT) and reduced with a single ones-matmul
     per (b, qchunk) - 4 den matmuls per head-batch instead of 22.
  4. Two AllToAlls (heads 0-1, then 2-3) exchange attn-out^T blocks.
  5. o_proj: out^T[hid, my_s_chunk] accumulated over all 32 feature tiles
     (Wo host-permuted into A2A block order, streamed), fp32 out.
Host reassembles the 8 sequence chunks and transposes back.
"""

import os

import numpy as np
import ml_dtypes

from concourse import bacc, mybir
import concourse.tile as tile
from concourse.bass_utils import run_bass_kernel_spmd

N_CORES = 8
B, S, HID = 2, 2048, 2048
H, HKV, D = 32, 8, 128
QH = H // HKV          # q heads per core
SG = B * S             # 4096 global sequence
NSC = SG // 512        # 8 s-chunks of 512
NKT = HID // 128       # 16 hid k-tiles
NFT = (H * D) // 128   # 32 o_proj contraction tiles

BF = mybir.dt.bfloat16
F32 = mybir.dt.float32
AF = mybir.ActivationFunctionType

_CACHE = {}
LAST_EXEC_NS = None
LAST_TRACE = None


def _build():
    nc = bacc.Bacc("TRN2", num_devices=N_CORES)

    hT_e = nc.declare_dram_parameter("hT", [HID, SG], BF, isOutput=False)
    wq_e = nc.declare_dram_parameter("wq", [HID, QH * D], BF, isOutput=False)
    wk_e = nc.declare_dram_parameter("wk", [HID, D], BF, isOutput=False)
    wv_e = nc.declare_dram_parameter("wv", [HID, D], BF, isOutput=False)
    # Wo host-permuted into per-(half, hid_t) contiguous [128, 16*128]
    # blocks in the order o_proj consumes them (few-descriptor DMAs).
    wo_e = nc.declare_dram_parameter("wo", [2 * NKT * 128, 16 * 128], BF,
                                     isOutput=False)
    cd_e = nc.declare_dram_parameter("c_dup", [D, SG], BF, isOutput=False)
    sd_e = nc.declare_dram_parameter("s_dup", [D, SG], BF, isOutput=False)
    id_e = nc.declare_dram_parameter("ident", [D, D], BF, isOutput=False)
    # duplicated diagonal mask: masks[kk, b*128+c] = (kk <= c)
    mk_e = nc.declare_dram_parameter("masks", [128, 256], BF, isOutput=False)
    outT_e = nc.declare_dram_parameter("outT", [HID, 512], F32, isOutput=True)

    with tile.TileContext(nc) as tc:
        with (
            tc.tile_pool(name="cst", bufs=1) as cst,
            tc.tile_pool(name="sb", bufs=2) as sb,
            tc.tile_pool(name="agp", bufs=1) as agp,
            tc.tile_pool(name="wop", bufs=8) as wop,
            tc.tile_pool(name="dram", bufs=1, space="DRAM") as dram,
        ):
            tril2 = cst.tile([128, 2, 128], BF, tag="tril2")
            nc.sync.dma_start(tril2[:], mk_e[:].rearrange("p (b c) -> p b c", b=2))
            ones_mat = cst.tile([128, 128], BF, tag="ones_mat")
            nc.gpsimd.memset(ones_mat[:], 1.0)

            qr = cst.tile([128, QH * SG], BF, tag="qr")
            kr = cst.tile([128, SG], BF, tag="kr")
            v_seq = cst.tile([128, SG], BF, tag="v_seq")

            # A2A bounce buffers: shard j = rows [j*256, (j+1)*256) =
            # (2 heads x 128d, s-chunk j's 512 cols).
            a2a_in = [
                dram.tile([8 * 256, 512], BF, name=f"a2ain{i}", tag=f"a2ain{i}")
                for i in (0, 1)
            ]
            a2a_out = [
                dram.tile([8 * 256, 512], BF, name=f"a2aout{i}", tag=f"a2aout{i}")
                for i in (0, 1)
            ]

            # ---- phase 1: projections + rope + V transpose ----
            with tc.tile_pool(name="p1", bufs=1) as p1, \
                 tc.tile_pool(name="htp", bufs=2) as htp, \
                 tc.tile_pool(name="pmm", bufs=6, space="PSUM") as pmm, \
                 tc.tile_pool(name="ptp", bufs=2, space="PSUM") as ptp:
                # wq on the sync HWDGE queue, the first h^T chunk on the
                # scalar HWDGE queue - two independent FIFOs so the first
                # projection chain streams as pieces land on both.
                wq_sb = p1.tile([128, NKT, QH * D], BF, tag="wq_sb")
                ht0 = htp.tile([128, NKT, 512], BF, tag="ht")
                # kt-granular pieces for the first 8 kt (lets the first
                # chain start within ~2us), coarser 4-kt pieces after.
                # Pieces alternate between the two HWDGE queues so the
                # first chain streams at 2x the single-queue issue rate.
                pieces = [(k, 1) for k in range(8)] + [(8, 4), (12, 4)]
                for pi, (k0, nk) in enumerate(pieces):
                    qa = nc.sync if pi % 2 == 0 else nc.scalar
                    qb = nc.scalar if pi % 2 == 0 else nc.sync
                    qa.dma_start(
                        wq_sb[:, k0 : k0 + nk, :],
                        wq_e[k0 * 128 : (k0 + nk) * 128, :].rearrange(
                            "(kt p) f -> p kt f", p=128
                        ),
                    )
                    qb.dma_start(
                        ht0[:, k0 : k0 + nk, :],
                        hT_e[k0 * 128 : (k0 + nk) * 128, 0:512].rearrange(
                            "(kt p) s -> p kt s", p=128
                        ),
                    )
                wk_sb = p1.tile([128, NKT, D], BF, tag="wk_sb")
                nc.sync.dma_start(
                    wk_sb[:], wk_e[:].rearrange("(kt p) f -> p kt f", p=128)
                )
                wv_sb = p1.tile([128, NKT, D], BF, tag="wv_sb")
                nc.sync.dma_start(
                    wv_sb[:], wv_e[:].rearrange("(kt p) f -> p kt f", p=128)
                )
                ident = p1.tile([D, D], BF, tag="ident")
                nc.sync.dma_start(ident[:], id_e[:])
                # rope tables via the gpsimd SWDGE queue (idle until rope)
                c_d = p1.tile([D, SG], BF, tag="c_d")
                nc.gpsimd.dma_start(c_d[:], cd_e[:])
                s_d = p1.tile([D, SG], BF, tag="s_d")
                nc.gpsimd.dma_start(s_d[:], sd_e[:])

                # rope/V-transpose for tile i are emitted AFTER projection
                # chain i+1 so their PE ops never wait on the ACT evacuation.
                def finish_tile(sc, ft, xb):
                    if ft < QH + 1:  # rope for q heads and k
                        # rotate-half = partition swap via idle gpsimd SWDGE
                        # (sin table sign-folded on host)
                        sh = sb.tile([128, 512], BF, tag="sh", bufs=3)
                        nc.gpsimd.dma_start(sh[0:64, :], xb[64:128, :])
                        nc.gpsimd.dma_start(sh[64:128, :], xb[0:64, :])
                        if ft < QH:
                            dest = qr[
                                :, ft * SG + sc * 512 : ft * SG + sc * 512 + 512
                            ]
                        else:
                            dest = kr[:, sc * 512 : sc * 512 + 512]
                        cs = c_d[:, sc * 512 : (sc + 1) * 512]
                        ss = s_d[:, sc * 512 : (sc + 1) * 512]
                        nc.vector.tensor_mul(dest, xb[:], cs)
                        rtmp = sb.tile([128, 512], BF, tag="rtmp")
                        nc.vector.tensor_mul(rtmp[:], sh[:], ss)
                        nc.vector.tensor_add(dest, dest, rtmp[:])
                    else:  # v: transpose to seq-major
                        for j in range(4):
                            tp = ptp.tile([128, 128], BF, tag="tp")
                            nc.tensor.transpose(
                                tp[:], xb[:, j * 128 : (j + 1) * 128], ident[:]
                            )
                            g = sc * 4 + j
                            nc.vector.tensor_copy(
                                v_seq[:, g * 128 : (g + 1) * 128], tp[:]
                            )

                def proj_lhsT(ft, kt):
                    if ft < QH:
                        return wq_sb[:, kt, ft * D : (ft + 1) * D]
                    if ft == QH:
                        return wk_sb[:, kt, :]
                    return wv_sb[:, kt, :]

                with nc.named_scope("proj"):
                    pending = None
                    for sc in range(NSC):
                        if sc == 0:
                            # kt-major: all 6 output chains advance per
                            # arriving kt piece, so the ramp is PE-paced
                            # (one piece feeds 6 matmuls) instead of
                            # DMA-paced.
                            ht = ht0
                            accs = [
                                pmm.tile([128, 512], F32, tag="mm",
                                         name=f"acc0_{ft}")
                                for ft in range(QH + 2)
                            ]
                            for kt in range(NKT):
                                for ft in range(QH + 2):
                                    nc.tensor.matmul(
                                        accs[ft][:], proj_lhsT(ft, kt),
                                        ht[:, kt, :],
                                        start=(kt == 0), stop=(kt == NKT - 1),
                                    )
                            for ft in range(QH + 2):
                                xb = sb.tile([128, 512], BF, tag="xb", bufs=4)
                                nc.scalar.activation(xb[:], accs[ft][:], AF.Copy)
                                if pending is not None:
                                    finish_tile(*pending)
                                pending = (sc, ft, xb)
                            continue
                        ht = htp.tile([128, NKT, 512], BF, tag="ht")
                        src = hT_e[:, sc * 512 : (sc + 1) * 512].rearrange(
                            "(kt p) s -> p kt s", p=128
                        )
                        # alternate HWDGE queues to halve per-queue load
                        if sc % 2:
                            nc.scalar.dma_start(ht[:], src)
                        else:
                            nc.sync.dma_start(ht[:], src)
                        for ft in range(QH + 2):  # 0..3 q heads, 4 k, 5 v
                            acc = pmm.tile([128, 512], F32, tag="mm")
                            for kt in range(NKT):
                                nc.tensor.matmul(
                                    acc[:], proj_lhsT(ft, kt), ht[:, kt, :],
                                    start=(kt == 0), stop=(kt == NKT - 1),
                                )
                            xb = sb.tile([128, 512], BF, tag="xb", bufs=4)
                            nc.scalar.activation(xb[:], acc[:], AF.Copy)
                            if pending is not None:
                                finish_tile(*pending)
                            pending = (sc, ft, xb)
                    finish_tile(*pending)

            # Prefetch the first 8 o_proj weight tiles NOW: these sync-queue
            # loads sit AHEAD of the attention a2a_in writes in the FIFO, so
            # they land during attention instead of head-of-line blocking
            # behind the last epilogue (only 8 = ring capacity; a 9th would
            # deadlock the queue against oproj progress).
            def load_wo(half, hid_t):
                wo_t = wop.tile([128, 16, 128], BF, tag="wo_t")
                r0 = (half * NKT + hid_t) * 128
                nc.sync.dma_start(
                    wo_t[:],
                    wo_e[r0 : r0 + 128, :].rearrange("p (ft c) -> p ft c", ft=16),
                )
                return wo_t

            wo_pre = [load_wo(0, hid_t) for hid_t in range(8)]

            # ---- phase 2: attention, b=0/b=1 in lockstep per (h, qc),
            # flattened into one software pipeline: score matmuls always run
            # 2 kt-tiles ahead ACROSS qc/h boundaries so the per-qc epilogue
            # (den->recip->normalize) never bubbles the PE (bubbles also
            # HAM-rethrottle the PE clock, doubling the cost).
            agts = []
            with nc.named_scope("attn"), \
                 tc.tile_pool(name="spair", bufs=2, space="PSUM") as spair, \
                 tc.tile_pool(name="denp", bufs=1, space="PSUM") as denp, \
                 tc.tile_pool(name="accp", bufs=1, space="PSUM") as accp, \
                 tc.tile_pool(name="sba", bufs=1) as sba:

                stream = []
                for half in range(2):
                    for h in (2 * half, 2 * half + 1):
                        for qc in range(4):
                            nkt = 4 * qc + 4
                            for kt in range(nkt):
                                stream.append((half, h, qc, kt, nkt))

                def qoff(qc, kt):
                    j = kt - 4 * qc
                    return j * 128 if j > 0 else 0

                sptiles = {}

                def emit_score(j):
                    _, h, qc, kt, _ = stream[j]
                    o = qoff(qc, kt)
                    sp = spair.tile(
                        [128, 2, 512], F32, tag="sp", name=f"s_{h}_{qc}_{kt}"
                    )
                    for b in range(2):
                        qs = h * SG + b * S + qc * 512
                        nc.tensor.matmul(
                            sp[:, b, : 512 - o],
                            kr[:, b * S + kt * 128 : b * S + (kt + 1) * 128],
                            qr[:, qs + o : qs + 512],
                        )
                    sptiles[j] = sp

                # Schedule: exp/og/den/recip run at the stream head (they
                # form the serial DVE/ACT chain), PV matmuls lag L items
                # behind, and the acc normalize (ao) happens right after
                # PV(k_last) with rb already computed - so the single acc
                # buffer is freed fast and the PE never head-of-line blocks
                # on the epilogue.
                L = 2
                pts = {}
                state = {}

                def emit_pv(i):
                    half, h, qc, kt, nkt = stream[i]
                    pT, o, w = pts.pop(i)
                    st = state[(h, qc)]
                    if kt == 0:
                        st[0] = accp.tile(
                            [128, 2, 512], F32, tag="accp", name=f"acc_{h}_{qc}"
                        )
                    acc = st[0]
                    for b in range(2):
                        g = b * 16 + kt
                        nc.tensor.matmul(
                            acc[:, b, o:512],
                            v_seq[:, g * 128 : (g + 1) * 128],
                            pT[:, b, :w],
                            start=(kt == 0), stop=(kt == nkt - 1),
                        )
                    if kt != nkt - 1:
                        return
                    rb = st[2]
                    ao = sba.tile([128, 2, 512], BF, tag="ao", bufs=3)
                    nc.vector.tensor_mul(ao[:], acc[:], rb[:])
                    hh = h % 2
                    for b in range(2):
                        sc = b * 4 + qc
                        nc.sync.dma_start(
                            a2a_in[half][
                                sc * 256 + hh * 128 : sc * 256 + (hh + 1) * 128, :
                            ],
                            ao[:, b, :],
                        )
                    del state[(h, qc)]
                    if hh == 1 and qc == 3:  # last (h, qc) of this half
                        nc.gpsimd.collective_compute(
                            "AllToAll",
                            mybir.AluOpType.bypass,
                            replica_groups=[list(range(N_CORES))],
                            ins=[a2a_in[half].opt()],
                            outs=[a2a_out[half].opt()],
                        )
                        # a2a_out loads emitted HERE so they sit in the
                        # gpsimd FIFO before the next half's collective
                        # trigger (which blocks the queue while waiting on
                        # its own input writes).
                        agt = agp.tile([128, 16, 512], BF, tag=f"ag{half}")
                        for fq in range(4):
                            nc.gpsimd.dma_start(
                                agt[:, 4 * fq : 4 * fq + 4, :],
                                a2a_out[half][
                                    4 * fq * 128 : (4 * fq + 4) * 128, :
                                ].rearrange("(ft p) s -> p ft s", p=128),
                            )
                        agts.append(agt)

                emitted = 0
                for i, (half, h, qc, kt, nkt) in enumerate(stream):
                    while emitted < min(i + 2, len(stream)):
                        emit_score(emitted)
                        emitted += 1
                    sp = sptiles.pop(i)
                    o = qoff(qc, kt)
                    w = 512 - o
                    pT = sba.tile([128, 2, 512], BF, tag="pT", bufs=5)
                    # one exp covers both batches (2D free AP, 2 banks)
                    nc.scalar.activation(pT[:, :, :w], sp[:, :, :w], AF.Exp)
                    if kt - 4 * qc >= 0:
                        # only the 128 diagonal cols need the mask
                        nc.vector.tensor_mul(
                            pT[:, :, :128], pT[:, :, :128], tril2[:]
                        )
                    if kt == 0:
                        og = sba.tile([128, 2, 512], BF, tag="og", bufs=2)
                        state[(h, qc)] = [None, og, None]
                    og = state[(h, qc)][1]
                    # denominator accumulation on the DVE (both b at once)
                    if kt == 0:
                        nc.vector.tensor_copy(og[:], pT[:])
                    else:
                        nc.vector.tensor_add(
                            og[:, :, o:512], og[:, :, o:512], pT[:, :, :w]
                        )
                    if kt == nkt - 1:
                        # den + recip now: they only depend on og (exp),
                        # not on PV, so rb is ready before ao needs it.
                        den = denp.tile(
                            [128, 2, 512], F32, tag="dp", name=f"den_{h}_{qc}"
                        )
                        nc.tensor.matmul(den[:, 0, :], ones_mat[:], og[:, 0, :])
                        nc.tensor.matmul(den[:, 1, :], ones_mat[:], og[:, 1, :])
                        # den rows are identical (all-ones stationary) ==
                        # already broadcast across partitions.
                        rb = sba.tile([128, 2, 512], F32, tag="rb", bufs=2)
                        nc.vector.reciprocal_approx_fast(rb[:], den[:])
                        state[(h, qc)][2] = rb
                    pts[i] = (pT, o, w)
                    if i >= L:
                        emit_pv(i - L)
                for i in range(len(stream) - L, len(stream)):
                    emit_pv(i)

            # ---- phase 3: o_proj for my s-chunk, all hidden columns.
            # Two passes: pass 0 (features from A2A1) accumulates to SBUF
            # partials while A2A2 is still in flight; pass 1 adds the rest.
            with nc.named_scope("oproj"), \
                 tc.tile_pool(name="po", bufs=4, space="PSUM") as po, \
                 tc.tile_pool(name="prt", bufs=1) as prt:
                parts = []
                for half in range(2):
                    agt = agts[half]
                    for hid_t in range(NKT):  # 16 tiles of 128 hidden cols
                        if half == 0 and hid_t < 8:
                            wo_t = wo_pre[hid_t]
                        else:
                            wo_t = load_wo(half, hid_t)
                        o_ps = po.tile([128, 512], F32, tag="mm")
                        for ft in range(16):
                            nc.tensor.matmul(
                                o_ps[:],
                                wo_t[:, ft, :],
                                agt[:, ft, :],
                                start=(ft == 0),
                                stop=(ft == 15),
                            )
                        if half == 0:
                            part = prt.tile(
                                [128, 512], F32, tag=f"part{hid_t}"
                            )
                            nc.scalar.activation(part[:], o_ps[:], AF.Copy)
                            parts.append(part)
                        else:
                            ob = sb.tile([128, 512], F32, tag="ob", bufs=3)
                            nc.vector.tensor_add(ob[:], o_ps[:], parts[hid_t][:])
                            nc.sync.dma_start(
                                outT_e[hid_t * 128 : (hid_t + 1) * 128, :], ob[:]
                            )

    nc.compile()
    return nc


def _prep(hidden_states, sin_table, cos_table, Wq, Wk, Wv, Wo):
    bf = ml_dtypes.bfloat16
    flat = np.asarray(hidden_states, np.float32).reshape(SG, HID)
    hT = np.ascontiguousarray(flat.T).astype(bf)

    cosT = np.asarray(cos_table, np.float32)[:, :64].T  # [64, S]
    sinT = np.asarray(sin_table, np.float32)[:, :64].T
    c_dup = np.tile(np.concatenate([cosT, cosT], 0), (1, B)).astype(bf)
    # sign-folded: rotate-half becomes a plain partition swap
    s_dup = np.tile(np.concatenate([-sinT, sinT], 0), (1, B)).astype(bf)

    ident = np.eye(D, dtype=np.float32).astype(bf)

    kk = np.arange(128)[:, None]
    cc = np.arange(128)[None, :]
    tri = (kk <= cc).astype(np.float32)
    masks = np.concatenate([tri, tri], axis=1).astype(bf)  # [128, 256]

    scale = np.float32(1.0 / np.sqrt(D))
    Wq = np.asarray(Wq, np.float32) * scale
    Wk = np.asarray(Wk, np.float32)
    Wv = np.asarray(Wv, np.float32)
    Wo = np.asarray(Wo, np.float32)

    # Permute Wo rows into the order o_proj consumes the A2A output blocks
    # (a2a1 blocks: (r, h in {0,1}); a2a2 blocks: (r, h in {2,3})), then
    # re-block so each (half, hid_t) wo_t tile is one contiguous
    # [128, 16*128] DRAM block: wo_r[half, hid_t, p, ft, c].
    Wo_b = Wo.reshape(H, D, HID)
    order = [4 * r + h for r in range(8) for h in (0, 1)] + [
        4 * r + h for r in range(8) for h in (2, 3)
    ]
    Wo_perm = Wo_b[order].reshape(H * D, HID)
    Wp = Wo_perm.reshape(2, 16, 128, NKT, 128)  # [half, ft, p, hid_t, c]
    Wo_r = np.ascontiguousarray(
        Wp.transpose(0, 3, 2, 1, 4).reshape(2 * NKT * 128, 16 * 128)
    ).astype(bf)

    in_maps = []
    for c in range(N_CORES):
        in_maps.append(
            {
                "hT": hT,
                "wq": np.ascontiguousarray(Wq[:, c * 512 : (c + 1) * 512]).astype(bf),
                "wk": np.ascontiguousarray(Wk[:, c * D : (c + 1) * D]).astype(bf),
                "wv": np.ascontiguousarray(Wv[:, c * D : (c + 1) * D]).astype(bf),
                "wo": Wo_r,
                "c_dup": c_dup,
                "s_dup": s_dup,
                "ident": ident,
                "masks": masks,
            }
        )
    return in_maps


def kernel(**inputs) -> np.ndarray:
    global LAST_EXEC_NS, LAST_TRACE
    if "nc" not in _CACHE:
        _CACHE["nc"] = _build()
    nc = _CACHE["nc"]

    in_maps = _prep(**inputs)
    res = run_bass_kernel_spmd(
        nc,
        in_maps,
        core_ids=list(range(N_CORES)),
        trace=bool(os.environ.get("BASS_TRACE")),
    )
    LAST_EXEC_NS = res.exec_time_ns
    LAST_TRACE = res.instructions_and_trace
    globals()["LAST_SCOPES"] = res.per_core_scope_times

    outT = np.concatenate(
        [np.asarray(res.results[c]["outT"], np.float32) for c in range(N_CORES)],
        axis=1,
    )  # [HID, SG]
    return np.ascontiguousarray(outT.T).reshape(B, S, HID)
